# revision 5
# baseline (speedup 1.0000x reference)
"""Causal self-attention Trainium2 kernel (v2).

Problem: B=4, T=2048, D=1024, H=16 heads, Dh=64.
Sharding: 8 cores = 4 batches x 2 head-groups (8 heads/group).
  - data parallel over batch, tensor parallel over heads
    (qkv column-parallel, out_proj row-parallel; host sums the two
    partial outputs per batch and adds the bias row).

Per-core kernel (Tile framework, bf16 matmuls with fp32 PSUM accum):
  phase 1: QKV projection.
      Q^T, K^T stored [head_dim, T] (pair-packed: 2 heads -> 128 parts)
      V stored [T, 8 heads x (64 dims + ones-col)] (ones col -> denom)
  attention per (q-tile of 512, head-pair), q-tiles DESCENDING:
      S^T[k,q] = K-block.T @ Q^T   (free dim trimmed at the causal diag)
      P^T = exp(S^T / 8)           (activation ranges trimmed too)
      causal: block skipping + one static 128x128 triangle mask per
      diagonal block
      O[q,d] (+ denom col) = P^T-chunk.T @ [V|1]  (q-chunked: 128-out
      partitions x 65-free moving — full PE-array utilization)
      normalize per q-partition: recip + tensor_scalar_mul (DVE)
      transpose O -> O^T via PE identity-transpose, assemble ot_sb
  proj: y_partial = O_norm @ W_out_rows (accumulate over head pairs)

  Emission order: attention q-tiles descending (3..0) per pair, with
  qkv-projection / V / out-projection psum-groups used as PE filler
  between attention units so the scalar-engine exp stream (the
  attention-phase bottleneck) stays hidden.
"""

import os
import sys

import numpy as np

sys.path.insert(0, "/opt/trn_rl_repo")

import ml_dtypes  # noqa: E402

import concourse.bass as bass  # noqa: E402
import concourse.mybir as mybir  # noqa: E402
import concourse.tile as tile  # noqa: E402
from concourse import bacc  # noqa: E402
from concourse.bass_utils import run_bass_kernel_spmd  # noqa: E402

BF16 = mybir.dt.bfloat16
F32 = mybir.dt.float32

B, T, D = 4, 2048, 1024
H, DH = 16, 64
G = 2                      # head groups (cores per batch)
HL = H // G                # heads per core (8)
CL = HL * DH               # local channel width (512)
NP = HL // 2               # head pairs per core (4)
QT = 512                   # query tile (free dim)
KB = 128                   # key block
NQT = T // QT              # 4
NTT = T // 128             # 16 t-tiles of 128
NDB = D // 128             # 8 contraction blocks for projections
SCALE = 1.0 / 8.0          # 1/sqrt(DH)

_CACHE: dict = {}

# cost-model constants for pacing (ns)
_PE_CYC = 1.0 / 2.4
_ACT_CYC = 1.0 / 1.2
_ACT_OVH = 260.0           # per-activation decode+access overhead estimate


def _build_program(reps=1):
    nc = bacc.Bacc(
        "TRN2",
        target_bir_lowering=False,
        debug=False,
        num_devices=8,
    )

    xT_d = nc.dram_tensor("xT", [D, T], BF16, kind="ExternalInput")
    wq_d = nc.dram_tensor("wq", [D, CL], BF16, kind="ExternalInput")
    wk_d = nc.dram_tensor("wk", [D, CL], BF16, kind="ExternalInput")
    wv_d = nc.dram_tensor("wv", [D, CL], BF16, kind="ExternalInput")
    wo_d = nc.dram_tensor("wo", [CL, D], BF16, kind="ExternalInput")
    bqk_d = nc.dram_tensor("bqk", [2, NP, 128, 1], F32, kind="ExternalInput")
    tri_d = nc.dram_tensor("tri", [128, 128], BF16, kind="ExternalInput")
    idn_d = nc.dram_tensor("idn", [128, 128], BF16, kind="ExternalInput")
    y_d = nc.dram_tensor("y", [T, D], F32, kind="ExternalOutput")

    Exp = mybir.ActivationFunctionType.Exp

    with tile.TileContext(nc) as tc:
        with (
            tc.tile_pool(name="const", bufs=1) as cpool,
            tc.tile_pool(name="big", bufs=1) as bpool,
            tc.tile_pool(name="pt", bufs=8) as ptpool,
            tc.tile_pool(name="nrm", bufs=3) as npool,
            tc.tile_pool(name="out", bufs=4) as opool,
            tc.tile_pool(name="ps_st", bufs=2, space="PSUM") as pst,
            tc.tile_pool(name="ps_o", bufs=2, space="PSUM") as pso,
            tc.tile_pool(name="ps_mm", bufs=2, space="PSUM") as pmm,
        ):
          # rep loop for steady-state HW timing (reps>1 only for bench)
          for rep in range(reps):
            # ---- persistent SBUF tiles -------------------------------------
            wq_sb = cpool.tile([128, NDB, CL], BF16, tag="wq", name="wq_sb")
            wk_sb = cpool.tile([128, NDB, CL], BF16, tag="wk", name="wk_sb")
            wv_sb = cpool.tile([128, NDB, CL], BF16, tag="wv", name="wv_sb")
            wo_sb = cpool.tile([128, NP, D], BF16, tag="wo", name="wo_sb")
            xT_sb = cpool.tile([128, NDB, T], BF16, tag="xt", name="xT_sb")
            tri_sb = cpool.tile([128, 128], BF16, tag="tri", name="tri_sb")
            idn_sb = cpool.tile([128, 128], BF16, tag="idn", name="idn_sb")
            bias_sb = cpool.tile([128, 2, NP], F32, tag="bias", name="bias_sb")

            qt_sb = [bpool.tile([128, T], BF16, tag=f"q{p}", name=f"q{p}")
                     for p in range(NP)]
            kt_sb = [bpool.tile([128, T], BF16, tag=f"k{p}", name=f"k{p}")
                     for p in range(NP)]
            v_sb = [bpool.tile([128, HL, DH + 1], BF16, tag=f"v{i}",
                               name=f"v{i}") for i in range(NTT)]
            ot_sb = [bpool.tile([128, T], BF16, tag=f"o{p}", name=f"o{p}")
                     for p in range(NP)]

            # ---- input DMAs (order = availability order) -------------------
            bqk_r = bqk_d[:].rearrange("w np p one -> p w (np one)")
            nc.sync.dma_start(bias_sb[:], bqk_r)
            nc.sync.dma_start(tri_sb[:], tri_d[:])
            nc.sync.dma_start(idn_sb[:], idn_d[:])
            wk_r = wk_d[:].rearrange("(db p) c -> p db c", p=128)
            nc.sync.dma_start(wk_sb[:], wk_r)
            xT_r = xT_d[:].rearrange("(db p) t -> p db t", p=128)
            for t4 in range(NQT):
                cs = slice(t4 * QT, (t4 + 1) * QT)
                nc.sync.dma_start(xT_sb[:, :, cs], xT_r[:, :, cs])
            wq_r = wq_d[:].rearrange("(db p) c -> p db c", p=128)
            nc.sync.dma_start(wq_sb[:], wq_r)
            wv_r = wv_d[:].rearrange("(db p) c -> p db c", p=128)
            nc.sync.dma_start(wv_sb[:], wv_r)
            wo_r = wo_d[:].rearrange("(np p) d -> p np d", p=128)
            nc.sync.dma_start(wo_sb[:], wo_r)

            # ---- filler groups (one psum-group each) -----------------------
            def emit_qk(which, p, t4):
                """Q (which=0) / K (which=1) projection group for (p, t4)."""
                wsb = wq_sb if which == 0 else wk_sb
                dst = qt_sb[p] if which == 0 else kt_sb[p]
                ps = pmm.tile([128, QT], F32, tag="mm",
                              name=f"ps_qk{which}_{p}_{t4}_{rep}")
                for db in range(NDB):
                    nc.tensor.matmul(
                        ps[:],
                        wsb[:, db, p * 128:(p + 1) * 128],
                        xT_sb[:, db, t4 * QT:(t4 + 1) * QT],
                        start=(db == 0),
                        stop=(db == NDB - 1),
                    )
                nc.vector.tensor_scalar_add(
                    dst[:, t4 * QT:(t4 + 1) * QT], ps[:],
                    bias_sb[:, which, p:p + 1],
                )

            def emit_v(tt):
                """V rows for t-tile tt -> v_sb[tt][:, h, 0:64]; col 64=1."""
                ps = pmm.tile([128, QT], F32, tag="mm", name=f"ps_v{tt}_{rep}")
                for db in range(NDB):
                    nc.tensor.matmul(
                        ps[:],
                        xT_sb[:, db, tt * 128:(tt + 1) * 128],
                        wv_sb[:, db, :],
                        start=(db == 0),
                        stop=(db == NDB - 1),
                    )
                nc.vector.memset(v_sb[tt][:, :, DH:DH + 1], 1.0)
                nc.vector.tensor_copy(
                    v_sb[tt][:, :, 0:DH],
                    ps[:].rearrange("p (h d) -> p h d", h=HL),
                )

            def emit_proj(qt, ct, tt):
                """Out-projection group: y rows tt*128.., cols ct*512.."""
                ps = pmm.tile([128, QT], F32, tag="mm",
                              name=f"ps_y{ct}_{tt}_{rep}")
                for p in range(NP):
                    nc.tensor.matmul(
                        ps[:],
                        ot_sb[p][:, tt * 128:(tt + 1) * 128],
                        wo_sb[:, p, ct * QT:(ct + 1) * QT],
                        start=(p == 0),
                        stop=(p == NP - 1),
                    )
                ysb = opool.tile([128, QT], F32, tag="ysb",
                                 name=f"ysb{ct}_{tt}_{rep}")
                nc.vector.tensor_copy(ysb[:], ps[:])
                nc.sync.dma_start(
                    y_d[tt * 128:(tt + 1) * 128, ct * QT:(ct + 1) * QT],
                    ysb[:],
                )

            # ---- filler scheduler ------------------------------------------
            # Each filler is (key, emit_fn, pe_ns). Consumed in order by
            # pace() during attention, or forced by key beforehand.
            fillers: list = []
            done: set = set()

            def add_filler(key, fn, pe_ns):
                fillers.append([key, fn, pe_ns])

            def force(key):
                for item in fillers:
                    if item[0] == key and key not in done:
                        done.add(key)
                        item[1]()
                        return item[2]
                return 0.0

            balance = [0.0]     # act_ns - attention pe_ns emitted so far

            def pace():
                while balance[0] > 0.0 and fillers:
                    while fillers and fillers[0][0] in done:
                        fillers.pop(0)
                    if not fillers:
                        return
                    key, fn, pe_ns = fillers.pop(0)
                    done.add(key)
                    fn()
                    balance[0] -= pe_ns

            QK_NS = 8 * QT * _PE_CYC
            V_NS = 8 * QT * _PE_CYC
            PJ_NS = 4 * QT * _PE_CYC

            # ---- attention for one (qt, head-pair) -------------------------
            def emit_attn(qt, p):
                nkb = 4 * (qt + 1)
                ps_o = [pso.tile([128, 4, DH + 1], F32, tag="o",
                                 name=f"ps_o{qt}_{p}_{h}_{rep}")
                        for h in range(2)]
                pt_prev = None

                def emit_pv(kbp, pts):
                    # ps_o[h] is one 2KB psum zero-region: exactly one
                    # start (kb0,c0) and one stop (last kb, c3) per tile.
                    for h in range(2):
                        for j in range(2):
                            kb = 2 * kbp + j
                            jd = kb - 4 * qt
                            for c in range(4):
                                if jd > c:
                                    continue
                                nc.tensor.matmul(
                                    ps_o[h][:, c, :],
                                    pts[h][:, j * QT + c * 128:
                                           j * QT + (c + 1) * 128],
                                    v_sb[kb][:, 2 * p + h, :],
                                    start=(kb == 0 and c == 0),
                                    stop=(kb == nkb - 1 and c == 3),
                                    skip_group_check=True,
                                )

                for kbp in range(nkb // 2):
                    # V prereqs for the PV of the PREVIOUS unit (kbs
                    # 2kbp-2, 2kbp-1) and, one unit ahead, this unit's.
                    for tt in range(min(2 * kbp + 2, nkb)):
                        force(("V", tt))
                    pts = []
                    for h in range(2):
                        rows = slice(64 * h, 64 * h + 64)
                        st = pst.tile([128, 2 * QT], F32, tag="st",
                                      name=f"st{qt}_{p}_{kbp}_{h}_{rep}")
                        lo = [0, 0]
                        for j in range(2):
                            kb = 2 * kbp + j
                            jd = kb - 4 * qt
                            lo[j] = 128 * jd if jd > 0 else 0
                            nc.tensor.matmul(
                                st[:, j * QT + lo[j]:(j + 1) * QT],
                                kt_sb[p][rows, kb * KB:(kb + 1) * KB],
                                qt_sb[p][rows,
                                         qt * QT + lo[j]:(qt + 1) * QT],
                                start=True, stop=True,
                            )
                        pt = ptpool.tile([128, 2 * QT], BF16, tag="pt",
                                         name=f"pt{qt}_{p}_{kbp}_{h}_{rep}")
                        acols = 0
                        for j in range(2):
                            a0, a1 = j * QT + lo[j], (j + 1) * QT
                            nc.scalar.activation(pt[:, a0:a1], st[:, a0:a1],
                                                 Exp, scale=SCALE)
                            acols += a1 - a0
                            balance[0] += _ACT_OVH
                        balance[0] += acols * _ACT_CYC
                        # triangle mask on the partial diagonal block
                        for j in range(2):
                            kb = 2 * kbp + j
                            jd = kb - 4 * qt
                            if 0 <= jd <= 3:
                                a0 = j * QT + 128 * jd
                                nc.vector.tensor_mul(
                                    pt[:, a0:a0 + 128], pt[:, a0:a0 + 128],
                                    tri_sb[:],
                                )
                        pts.append(pt)
                        balance[0] -= (2 * QT - lo[0] - lo[1]) * _PE_CYC
                    if pt_prev is not None:
                        emit_pv(kbp - 1, pt_prev)
                        npv = sum(4 - max(0, 2 * (kbp - 1) + j - 4 * qt)
                                  for j in range(2))
                        balance[0] -= 2 * npv * 65 * _PE_CYC
                    pt_prev = pts
                    pace()
                emit_pv(nkb // 2 - 1, pt_prev)

                # normalize: O[q,d]/denom[q], then transpose to O^T in ot_sb
                otq = npool.tile([128, 4, 128], BF16, tag="otq",
                                 name=f"otq{qt}_{p}_{rep}")
                for h in range(2):
                    rden = npool.tile([128, 4], F32, tag="rden",
                                      name=f"rden{qt}_{p}_{h}_{rep}")
                    nc.vector.reciprocal(rden[:], ps_o[h][:, :, DH])
                    for c in range(4):
                        nc.vector.tensor_scalar_mul(
                            otq[:, c, 64 * h:64 * h + 64],
                            ps_o[h][:, c, 0:DH],
                            rden[:, c:c + 1],
                        )
                tr = pmm.tile([128, 4, 128], BF16, tag="mm",
                              name=f"tr{qt}_{p}_{rep}")
                for c in range(4):
                    nc.tensor.matmul(tr[:, c, :], otq[:, c, :], idn_sb[:],
                                     start=(c == 0), stop=(c == 3),
                                     is_transpose=True,
                                     skip_group_check=True)
                nc.vector.tensor_copy(
                    ot_sb[p][:, qt * QT:(qt + 1) * QT],
                    tr[:].rearrange("p c q -> p (c q)"),
                )
                balance[0] -= 4 * 128 * _PE_CYC

            # ---- global schedule -------------------------------------------
            # seed: K(p0) fully + Q(p0, t4=3) emitted directly; the rest of
            # phase 1 + out-projection groups become paced fillers.
            for t4 in range(NQT):
                emit_qk(1, 0, t4)
                done.add(("K", 0, t4))
            emit_qk(0, 0, NQT - 1)
            done.add(("Q", 0, NQT - 1))

            for tt in range(NTT):
                add_filler(("V", tt), (lambda tt=tt: emit_v(tt)), V_NS)
            for t4 in range(NQT - 1):
                add_filler(("Q", 0, t4), (lambda t4=t4: emit_qk(0, 0, t4)),
                           QK_NS)
            for p in range(1, NP):
                for t4 in range(NQT):
                    add_filler(("K", p, t4),
                               (lambda p=p, t4=t4: emit_qk(1, p, t4)), QK_NS)
                for t4 in (NQT - 1, 0, 1, 2):
                    add_filler(("Q", p, t4),
                               (lambda p=p, t4=t4: emit_qk(0, p, t4)), QK_NS)

            for qt in range(NQT - 1, -1, -1):
                for p in range(NP):
                    for t4 in range(qt + 1):
                        force(("K", p, t4))
                    force(("Q", p, qt))
                    emit_attn(qt, p)
                if qt > 0:
                    for ct in range(2):
                        for tt in range(4 * qt, 4 * qt + 4):
                            add_filler(("PJ", qt, ct, tt),
                                       (lambda qt=qt, ct=ct, tt=tt:
                                        emit_proj(qt, ct, tt)), PJ_NS)
            # tail: leftovers + final projection
            for key, fn, _ in fillers:
                if key not in done:
                    done.add(key)
                    fn()
            for ct in range(2):
                for tt in range(4):
                    emit_proj(0, ct, tt)

    nc.compile()
    return nc


def _get_program():
    if "nc" not in _CACHE:
        _CACHE["nc"] = _build_program()
    return _CACHE["nc"]


def _tri_mask():
    """tri[k', u] = 1.0 if k' <= u else 0 (bf16, 128x128)."""
    kk = np.arange(128)[:, None]
    uu = np.arange(128)[None, :]
    return (kk <= uu).astype(ml_dtypes.bfloat16)


def make_in_maps(x, w_qkv, b_qkv, w_out):
    bf16 = ml_dtypes.bfloat16
    tri = _tri_mask()
    idn = np.eye(128, dtype=np.float32).astype(bf16)
    in_maps = []
    for c in range(8):
        b, g = c // 2, c % 2
        cs = slice(CL * g, CL * (g + 1))
        bq = b_qkv[cs.start:cs.stop].reshape(NP, 128, 1).astype(np.float32)
        bk = b_qkv[D + cs.start:D + cs.stop].reshape(NP, 128, 1).astype(np.float32)
        in_maps.append({
            "xT": np.ascontiguousarray(x[b].T).astype(bf16),
            "wq": np.ascontiguousarray(w_qkv[:, cs]).astype(bf16),
            "wk": np.ascontiguousarray(w_qkv[:, D + cs.start:D + cs.stop]).astype(bf16),
            "wv": np.ascontiguousarray(
                w_qkv[:, 2 * D + cs.start:2 * D + cs.stop]).astype(bf16),
            "wo": np.ascontiguousarray(w_out[cs, :]).astype(bf16),
            "bqk": np.stack([bq, bk]).astype(np.float32),
            "tri": tri,
            "idn": idn,
        })
    return in_maps


def kernel(x, w_qkv, b_qkv, w_out, b_out, _results_hook=None):
    x = np.asarray(x, dtype=np.float32)
    w_qkv = np.asarray(w_qkv, dtype=np.float32)
    b_qkv = np.asarray(b_qkv, dtype=np.float32)
    w_out = np.asarray(w_out, dtype=np.float32)
    b_out = np.asarray(b_out, dtype=np.float32)

    nc = _get_program()
    in_maps = make_in_maps(x, w_qkv, b_qkv, w_out)
    res = run_bass_kernel_spmd(nc, in_maps, list(range(8)))
    if _results_hook is not None:
        _results_hook(res)

    # host-side constant row: v-bias passes through softmax untouched
    # (attention rows sum to 1), then through the out projection.
    host_row = (
        b_qkv[2 * D:].astype(np.float64) @ w_out.astype(np.float64)
        + b_out.astype(np.float64)
    ).astype(np.float32)

    y = np.empty((B, T, D), dtype=np.float32)
    for b in range(B):
        y[b] = res.results[2 * b]["y"] + res.results[2 * b + 1]["y"] + host_row
    return y
